# revision 28
# baseline (speedup 1.0000x reference)
"""Neural CDE forward on 8 Trainium2 cores — GBS(2,4) extrapolated midpoint.

Replaces the reference's 2x-RK4-substep integration (8 serial vector-field
evals per knot interval) with a Gragg smoothed-midpoint pair (n=2, n=4) and
Richardson extrapolation: T = (4*T4 - T2)/3.  Same order, 7 distinct evals
per interval, and the two sequences are independent so their evals pack into
5 PE "ticks" per interval, two of them 128 columns wide:

  t0 (64):  f0 = g(z0, 0)         shared by both sequences
  t1 (128): a1 = g(e1A, .5)   |   b1 = g(e1B, .25)
  t2 (128): a2 = g(e2A, 1)    |   b2 = g(e2B, .5)
  t3 (64):  b3 = g(e3B, .75)
  t4 (64):  b4 = g(e4B, 1)

All state is feature-major [64 hid, batch] f16 so each tick's MLP streams
packed columns through the PE with no transposes.  y accumulates in six
fp32 PSUM bank regions; paired ticks put batch on the partition dim
(p = A|B), solo ticks pack the two h-halves there (p = half*64+b) so their
einsum still uses all 128 lanes at half the columns.  The tanh/mult/
segmented-reduce pipeline drains region by region behind the W_out streams;
k returns to feature-major via eperm permutation matmuls, in two 32-h
slices on paired ticks so state updates start before the drain finishes.
The interval-end extrapolation is folded to a single chained op
(z0' = kB4/6 + [(2/3)(e3B+e4B) + U6]), and the t0/t1 state-update scales
(all powers of two, f16-exact) are folded into pre-scaled eperm constants
so one broadcast scalar_tensor_tensor writes both sequence halves at once.
Measured on hw: rel err 8.98e-3 (gate 2e-2), ~1.062 ms fast-clock.
"""

import os
import numpy as np

B, T, IN, HID, HH, NCLS = 512, 16, 41, 64, 150, 4
NL = 3
NCORES = 8
BL = B // NCORES            # 64 per-core batch
INP = IN + 1                # 42: i padded so segments stay 4B-aligned
NIV = T - 1                 # 15 intervals
NTICK = 5
YC = HID * INP              # 2688 y columns
# fp32 PSUM regions: cols (hc*42) <= 512 fp32 per bank; last region small so
# the final tanh/mult/reduce tail is short.
REGIONS = [(0, 12), (12, 12), (24, 8), (32, 12), (44, 12), (56, 8)]
# solo ticks pack the two h-halves into partitions (p = half*64+b) so the
# einsum runs [128, 1344] instead of [64, 2688]; regions are h_local ranges.
# Solo drain is chain-paced (not DVE-bound), so the LAST region is small.
SOLO_REG = [(0, 12), (12, 12), (24, 8)]
# paired-tick h-slices for the pipelined tail; 32-partition aligned
# (matmul out.base_partition must be 0/32/64).
SLICES = [(0, 32, 2), (32, 32, 5)]  # (h0, hc, after-region)
# drain-engine split: the dX mult of EARLY regions goes to gpsimd (Pool,
# otherwise idle); late (chain-critical) regions stay on the faster DVE.
POOL_MULT_MAX = -1
SOLO_POOL_MULT_MAX = -1
# reduces via DVE pool_avg (one InstPool per region, sum/42 folded into dxh)
# — rejected by walrus codegen ("Pool input AP must have 5 dimensions")
USE_POOL_AVG = False
TICK_DX = [
    [0.0, 0.0],
    [0.5, 0.25],
    [1.0, 0.5],
    [0.75, 0.75],
    [1.0, 1.0],
]
TICK_ROWS = [64, 128, 128, 64, 64]
HEAT_MLP = int(os.environ.get("NCDE_HEAT_MLP", "0"))   # heaters before a layer
HEAT_POST = int(os.environ.get("NCDE_HEAT", "0"))      # heaters after W_out
HEAT_COLS = int(os.environ.get("NCDE_HEAT_COLS", "384"))


def _prep_shared(W0, b0, W_in, b_in, W_h, b_h, W_out, b_out, Wc1, bc1, Wc2, bc2):
    f16, f32 = np.float16, np.float32
    whA = np.concatenate([W_h[i][0:128, :] for i in range(NL)], axis=1)
    whB = np.concatenate([W_h[i][128:HH, :] for i in range(NL)], axis=1)
    bias_a = np.stack([b_in[0:128]] + [b_h[i][0:128] for i in range(NL)], axis=1)
    bias_b = np.stack([b_in[128:HH]] + [b_h[i][128:HH] for i in range(NL)], axis=1)
    Rp = np.zeros((HH, HID, INP), np.float32)
    Rp[:, :, :IN] = W_out.reshape(HH, HID, IN)
    W2 = Rp.reshape(HH, YC)
    bo = np.zeros((HID, INP), np.float32)
    bo[:, :IN] = b_out.reshape(HID, IN)
    return {
        "w0": W0.astype(f16),
        "b0c": b0.reshape(HID, 1).astype(f32),
        "wiA": W_in[:, 0:128].astype(f16),
        "wiB": W_in[:, 128:HH].astype(f16),
        "whA": whA.astype(f16),
        "whB": whB.astype(f16),
        "bias_a": bias_a.astype(f32),
        "bias_b": bias_b.astype(f32),
        "woa": W2[0:128].astype(f16),
        "wob": np.vstack([W2[128:HH], bo.reshape(1, YC)]).astype(f16),
        "wc1": Wc1.astype(f16),
        "bc1c": bc1.reshape(HID, 1).astype(f32),
        "wc2": Wc2.astype(f16),
        "bc2c": bc2.reshape(NCLS, 1).astype(f32),
        "id128": np.eye(128, dtype=np.float16),
        "eperm": np.vstack([
            np.hstack([np.eye(64), 0 * np.eye(64)]),
            np.hstack([0 * np.eye(64), np.eye(64)]),
        ]).astype(np.float16),
        "eperm2": np.vstack([
            np.hstack([np.eye(64), 0 * np.eye(64)]),
            np.hstack([0 * np.eye(64), 0.5 * np.eye(64)]),
        ]).astype(np.float16),
        "ep0a": np.hstack([
            0.5 * np.vstack([np.eye(64), 0 * np.eye(64)]),
            0.25 * np.vstack([np.eye(64), 0 * np.eye(64)]),
        ]).astype(np.float16),
        "ep0b": np.hstack([
            0.5 * np.vstack([0 * np.eye(64), np.eye(64)]),
            0.25 * np.vstack([0 * np.eye(64), np.eye(64)]),
        ]).astype(np.float16),
        "hhB_init": np.vstack(
            [np.zeros((HH - 128, BL * 2), np.float32), np.ones((1, BL * 2), np.float32)]
        ).astype(f16),
    }


def _prep_percore(bc_core):
    x0t = bc_core[:, 0, 0, :].T.astype(np.float16)
    c1, c2, c3 = bc_core[:, :, 1, :], bc_core[:, :, 2, :], bc_core[:, :, 3, :]
    dxh = np.zeros((128, NIV * NTICK * INP), np.float32)
    for iv in range(NIV):
        for t in range(NTICK):
            col = (iv * NTICK + t) * INP
            for half, s in enumerate(TICK_DX[t]):
                dX = c1[:, iv] + (2.0 * s) * c2[:, iv] + (3.0 * s * s) * c3[:, iv]
                dxh[half * BL:(half + 1) * BL, col:col + IN] = dX
    if USE_POOL_AVG:
        # k comes out of pool_avg (sum/INP); pre-scale dX so avg(pr) == k
        dxh *= float(INP)
    return x0t, dxh.astype(np.float16)


def build_nc():
    from contextlib import ExitStack

    import concourse.bass as bass
    import concourse.mybir as mybir
    from concourse import bacc, tile

    f16 = mybir.dt.float16
    f32 = mybir.dt.float32
    AF = mybir.ActivationFunctionType
    OP = mybir.AluOpType

    nc = bacc.Bacc("TRN2", target_bir_lowering=False, debug=False)

    dram = {}
    ins_spec = [
        ("x0t", [IN, BL], f16),
        ("dxh", [128, NIV * NTICK * INP], f16),
        ("w0", [IN, HID], f16),
        ("b0c", [HID, 1], f32),
        ("wiA", [HID, 128], f16),
        ("wiB", [HID, HH - 128], f16),
        ("whA", [128, NL * HH], f16),
        ("whB", [HH - 128, NL * HH], f16),
        ("bias_a", [128, 1 + NL], f32),
        ("bias_b", [HH - 128, 1 + NL], f32),
        ("woa", [128, YC], f16),
        ("wob", [HH - 128 + 1, YC], f16),
        ("wc1", [HID, HID], f16),
        ("bc1c", [HID, 1], f32),
        ("wc2", [HID, NCLS], f16),
        ("bc2c", [NCLS, 1], f32),
        ("id128", [128, 128], f16),
        ("eperm", [128, 128], f16),
        ("eperm2", [128, 128], f16),
        ("ep0a", [128, 128], f16),
        ("ep0b", [128, 128], f16),
        ("hhB_init", [HH - 128 + 1, 2 * BL], f16),
    ]
    for name, shape, dt in ins_spec:
        dram[name] = nc.dram_tensor(name, shape, dt, kind="ExternalInput")
    out_dram = nc.dram_tensor("pred_t", [NCLS, BL], f32, kind="ExternalOutput")

    with tile.TileContext(nc) as tc:
        with ExitStack() as ctx:
            const = ctx.enter_context(tc.tile_pool(name="const", bufs=1))
            work = ctx.enter_context(tc.tile_pool(name="work", bufs=2))
            st = ctx.enter_context(tc.tile_pool(name="st", bufs=1))
            ty_pool = ctx.enter_context(tc.tile_pool(name="ty", bufs=2))
            pr_pool = ctx.enter_context(tc.tile_pool(name="pr", bufs=2))
            ps_y = ctx.enter_context(
                tc.tile_pool(name="ps_y", bufs=1, space=bass.MemorySpace.PSUM)
            )
            ps_mlp = ctx.enter_context(
                tc.tile_pool(name="ps_mlp", bufs=1, space=bass.MemorySpace.PSUM)
            )
            ps_k = ctx.enter_context(
                tc.tile_pool(name="ps_k", bufs=1, space=bass.MemorySpace.PSUM)
            )

            sb = {}
            for name, shape, dt in ins_spec:
                tt = const.tile(shape, dt, tag=name, name=name)
                nc.sync.dma_start(tt[:], dram[name][:])
                sb[name] = tt

            yR = [
                ps_y.tile([128, 512], f32, tag=f"yR{r}", name=f"yR{r}")
                for r in range(len(REGIONS))
            ]
            pAB = ps_mlp.tile([128, 256], f32, tag="pAB", name="pAB")
            ktp = ps_k.tile([HID, 2 * BL], f32, tag="ktp", name="ktp")

            z0 = st.tile([HID, BL], f16, tag="z0", name="z0")
            zin1 = st.tile([HID, 2 * BL], f16, tag="zin1", name="zin1")
            zin2 = st.tile([HID, 2 * BL], f16, tag="zin2", name="zin2")
            zin3 = st.tile([HID, BL], f16, tag="zin3", name="zin3")
            zin4 = st.tile([HID, BL], f16, tag="zin4", name="zin4")
            S_t = st.tile([HID, BL], f32, tag="S", name="S_t")
            U6a = st.tile([HID, BL], f32, tag="U6a", name="U6a")
            U6 = st.tile([HID, BL], f32, tag="U6", name="U6")
            V_t = st.tile([HID, BL], f32, tag="V", name="V_t")
            Wt = st.tile([HID, BL], f32, tag="Wt", name="Wt")
            hhB = sb["hhB_init"]

            nc.tensor.matmul(pAB[0:HID, 0:BL], sb["w0"][:], sb["x0t"][:])
            nc.vector.tensor_scalar(z0[:], pAB[0:HID, 0:BL], sb["b0c"][:], None, OP.add)

            def heat(n, rbias=0):
                # heater matmuls: no live data deps (target y banks whose
                # tanh read has already issued), keep the PE p-state warm
                for i in range(n):
                    nc.tensor.matmul(
                        yR[(i + rbias) % len(REGIONS)][0:128, 0:HEAT_COLS],
                        sb["id128"][:],
                        sb["woa"][:, 0:HEAT_COLS],
                        skip_group_check=True,
                    )

            def tick(zin_ap, rows, dxcol, upd, sliced_in, kb_half=False,
                     t0_scale=False):
                """One packed vector-field eval.  After each h-slice's reduce,
                transposes that slice of k and calls upd(h0, hc, kT_slice)."""
                hA = None
                for layer in range(1 + NL):
                    heat(HEAT_MLP, rbias=2 * layer)
                    if layer == 0:
                        if sliced_in:
                            # c-sliced accumulation so layer 0 overlaps the
                            # previous tick's einsum drain.  Groups must be
                            # sequential per bank: all of A, then all of B
                            # (interleaved open groups corrupt PSUM on hw).
                            for g, (h0, hc, _) in enumerate(SLICES):
                                nc.tensor.matmul(
                                    pAB[0:128, 0:rows],
                                    sb["wiA"][h0:h0 + hc, :],
                                    zin_ap[h0:h0 + hc, :],
                                    start=g == 0, stop=g == len(SLICES) - 1,
                                    skip_group_check=True,
                                )
                            for g, (h0, hc, _) in enumerate(SLICES):
                                nc.tensor.matmul(
                                    pAB[0:HH - 128, 128:128 + rows],
                                    sb["wiB"][h0:h0 + hc, :],
                                    zin_ap[h0:h0 + hc, :],
                                    start=g == 0, stop=g == len(SLICES) - 1,
                                    skip_group_check=True,
                                )
                        else:
                            nc.tensor.matmul(
                                pAB[0:128, 0:rows], sb["wiA"][:], zin_ap
                            )
                            nc.tensor.matmul(
                                pAB[0:HH - 128, 128:128 + rows], sb["wiB"][:], zin_ap
                            )
                    else:
                        c0 = (layer - 1) * HH
                        nc.tensor.matmul(
                            pAB[0:128, 0:rows],
                            sb["whA"][:, c0:c0 + 128],
                            hA[:, 0:rows],
                            start=True, stop=False, skip_group_check=True,
                        )
                        nc.tensor.matmul(
                            pAB[0:128, 0:rows],
                            sb["whB"][:, c0:c0 + 128],
                            hhB[0:HH - 128, 0:rows],
                            start=False, stop=True, skip_group_check=True,
                        )
                        nc.tensor.matmul(
                            pAB[0:HH - 128, 128:128 + rows],
                            sb["whA"][:, c0 + 128:c0 + HH],
                            hA[:, 0:rows],
                            start=True, stop=False, skip_group_check=True,
                        )
                        nc.tensor.matmul(
                            pAB[0:HH - 128, 128:128 + rows],
                            sb["whB"][:, c0 + 128:c0 + HH],
                            hhB[0:HH - 128, 0:rows],
                            start=False, stop=True, skip_group_check=True,
                        )
                    nhA = work.tile([128, 2 * BL], f16, tag="hA", name="nhA")
                    ba = sb["bias_a"][:, layer:layer + 1]
                    bb = sb["bias_b"][:, layer:layer + 1]
                    # B-half relu (DVE) emitted FIRST: the hw trace showed it
                    # serializing AFTER the ACT relu when emitted second,
                    # adding ~370ns to every layer hop
                    nc.vector.tensor_scalar(
                        hhB[0:HH - 128, 0:rows],
                        pAB[0:HH - 128, 128:128 + rows],
                        bb, 0.0, OP.add, OP.max,
                    )
                    nc.scalar.activation(
                        nhA[:, 0:rows], pAB[0:128, 0:rows], AF.Relu, bias=ba
                    )
                    hA = nhA

                paired = rows == 128
                regs = REGIONS if paired else SOLO_REG
                # einsum grouped by stationary operand (hA for woa, hhB for
                # wob) so the PE's weight regs reload only once per group —
                # kills the ~140ns LDW transition bubble between regions.
                if paired:
                    for r, (h0, hc) in enumerate(regs):
                        nc.tensor.matmul(
                            yR[r][0:128, 0:hc * INP],
                            hA[:, 0:rows],
                            sb["woa"][:, h0 * INP:(h0 + hc) * INP],
                            start=True, stop=False, skip_group_check=True,
                        )
                    for r, (h0, hc) in enumerate(regs):
                        nc.tensor.matmul(
                            yR[r][0:128, 0:hc * INP],
                            hhB[:, 0:rows],
                            sb["wob"][:, h0 * INP:(h0 + hc) * INP],
                            start=False, stop=True, skip_group_check=True,
                        )
                else:
                    for r, (h0, hc) in enumerate(regs):
                        for half in range(2):
                            c0 = (half * 32 + h0) * INP
                            nc.tensor.matmul(
                                yR[r][half * 64:half * 64 + 64, 0:hc * INP],
                                hA[:, 0:rows],
                                sb["woa"][:, c0:c0 + hc * INP],
                                start=True, stop=False, skip_group_check=True,
                            )
                    for r, (h0, hc) in enumerate(regs):
                        for half in range(2):
                            c0 = (half * 32 + h0) * INP
                            nc.tensor.matmul(
                                yR[r][half * 64:half * 64 + 64, 0:hc * INP],
                                hhB[:, 0:rows],
                                sb["wob"][:, c0:c0 + hc * INP],
                                start=False, stop=True, skip_group_check=True,
                            )
                # per-slice k tiles (reduced at free offset 0)
                k_sl = [
                    work.tile([128, 48], f16, tag=f"k_t{i}", name="k_sl")
                    for i in range(2)
                ]
                slice_i = 0
                for r, (h0, hc) in enumerate(regs):
                    ty = ty_pool.tile([128, 1024], f16, tag=f"ty{r % 2}", name="ty")
                    nc.scalar.activation(
                        ty[0:128, 0:hc * INP], yR[r][0:128, 0:hc * INP], AF.Tanh
                    )
                    pr = pr_pool.tile([128, 1024], f16, tag=f"pr{r % 2}", name="pr")
                    dxv = (
                        sb["dxh"][0:128, dxcol:dxcol + INP]
                        .unsqueeze(1)
                        .broadcast_to((128, hc, INP))
                    )
                    tyv = ty[0:128, 0:hc * INP].rearrange("p (h i) -> p h i", i=INP)
                    prv = pr[0:128, 0:hc * INP].rearrange("p (h i) -> p h i", i=INP)
                    pm = POOL_MULT_MAX if paired else SOLO_POOL_MULT_MAX
                    if r <= pm:
                        # early regions' mult on the otherwise-idle gpsimd
                        nc.gpsimd.tensor_tensor(prv, tyv, dxv, OP.mult)
                    else:
                        nc.vector.tensor_tensor(prv, tyv, dxv, OP.mult)
                    ksl = k_sl[slice_i] if paired else k_sl[0]
                    koff = SLICES[slice_i][0] if paired else 0
                    if USE_POOL_AVG:
                        nc.vector.pool(
                            ksl[0:128, h0 - koff:h0 - koff + hc],
                            prv, mybir.PoolFunctionType.avg,
                        )
                    else:
                        with nc.allow_low_precision(
                            reason="f16 rounding of final k"
                        ):
                            nc.vector.tensor_reduce(
                                ksl[0:128, h0 - koff:h0 - koff + hc],
                                prv, mybir.AxisListType.X, OP.add,
                            )
                    if paired and SLICES[slice_i][2] == r:
                        # k^T slice via ONE eperm matmul (block-diagonal
                        # permutation): ktp[s0:s0+32, 0:64]=kA, [..,64:128]=kB
                        # (eperm2's cols 0:64 equal eperm's, so one 128-col
                        # rhs covers both halves)
                        s0, sc, _ = SLICES[slice_i]
                        nc.tensor.matmul(
                            ktp[s0:s0 + sc, 0:2 * BL],
                            k_sl[slice_i][:, 0:sc],
                            (sb["eperm2"] if kb_half else sb["eperm"])
                            [:, 0:128],
                        )
                        upd(s0, sc)
                        slice_i += 1
                if not paired:
                    # k_sl[0][p=(half,b), h_local] -> ktp[h, b] f32; for t0
                    # the double-width scaled eperms emit [0.5*k | 0.25*k]
                    if t0_scale:
                        nc.tensor.matmul(
                            ktp[0:32, 0:2 * BL], k_sl[0][:, 0:32],
                            sb["ep0a"][:, 0:128],
                        )
                        nc.tensor.matmul(
                            ktp[32:64, 0:2 * BL], k_sl[0][:, 0:32],
                            sb["ep0b"][:, 0:128],
                        )
                    else:
                        nc.tensor.matmul(
                            ktp[0:32, 0:BL], k_sl[0][:, 0:32], sb["eperm"][:, 0:64]
                        )
                        nc.tensor.matmul(
                            ktp[32:64, 0:BL], k_sl[0][:, 0:32],
                            sb["eperm"][:, 64:128],
                        )
                    upd(0, 64)

            STT = nc.vector.scalar_tensor_tensor

            for iv in range(NIV):
                base = iv * NTICK * INP
                kA = lambda s0, sc: ktp[s0:s0 + sc, 0:BL]
                kB = lambda s0, sc: ktp[s0:s0 + sc, BL:2 * BL]

                def upd0(s0, sc):
                    # ktp holds [0.5*k | 0.25*k]; one broadcast STT writes
                    # both e1A and e1B
                    sl = slice(s0, s0 + sc)
                    o3 = zin1[sl, 0:2 * BL].rearrange("p (g b) -> p g b", b=BL)
                    k3 = ktp[sl, 0:2 * BL].rearrange("p (g b) -> p g b", b=BL)
                    z3 = z0[sl, :].unsqueeze(1).broadcast_to((sc, 2, BL))
                    STT(o3, k3, 1.0, z3, OP.mult, OP.add)

                tick(z0[:], 64, base, upd0, sliced_in=False, t0_scale=True)

                def upd1(s0, sc):
                    # ktp holds [kA | 0.5*kB] (eperm2); t2's layer-0 is
                    # whole-tile-gated, so one full-range broadcast STT after
                    # the final slice covers everything
                    if s0 == 0:
                        return
                    o3 = zin2[0:HID, 0:2 * BL].rearrange("p (g b) -> p g b", b=BL)
                    k3 = ktp[0:HID, 0:2 * BL].rearrange("p (g b) -> p g b", b=BL)
                    z3 = z0[0:HID, :].unsqueeze(1).broadcast_to((HID, 2, BL))
                    STT(o3, k3, 1.0, z3, OP.mult, OP.add)

                tick(zin1[:], 128, base + 1 * INP, upd1, sliced_in=False,
                     kb_half=True)

                def upd2(s0, sc):
                    if s0 == 0:
                        return
                    STT(zin3[0:HID, :], kB(0, HID), 0.5,
                        zin1[0:HID, BL:2 * BL], OP.mult, OP.add)

                tick(zin2[:], 128, base + 2 * INP, upd2, sliced_in=False)
                # off-chain A-branch combine: U6 = -(e1A+e2A+.5kA2)/6
                # (S_t/U6 on gpsimd; U6a reads ktp=PSUM so it stays on DVE)
                nc.gpsimd.tensor_tensor(S_t[:], zin1[:, 0:BL], zin2[:, 0:BL], OP.add)
                STT(U6a[:], ktp[0:HID, 0:BL], 0.5, S_t[:], OP.mult, OP.add)
                nc.vector.tensor_scalar(U6[:], U6a[:], -1.0 / 6.0, None, OP.mult)

                def upd3(s0, sc):
                    sl = slice(s0, s0 + sc)
                    STT(zin4[sl, :], kA(s0, sc), 0.5, zin2[sl, BL:2 * BL], OP.mult, OP.add)

                tick(zin3[:], 64, base + 3 * INP, upd3, sliced_in=False)
                # z0' = (4*T4 - T2)/3 = kB4/6 + VU,  VU = (2/3)(e3B+e4B) + U6
                nc.gpsimd.tensor_tensor(V_t[:], zin3[:], zin4[:], OP.add)
                STT(Wt[:], V_t[:], 2.0 / 3.0, U6[:], OP.mult, OP.add)

                def upd4(s0, sc):
                    sl = slice(s0, s0 + sc)
                    STT(z0[sl, :], kA(s0, sc), 1.0 / 6.0, Wt[sl, :], OP.mult, OP.add)

                tick(zin4[:], 64, base + 4 * INP, upd4, sliced_in=False)

            nc.tensor.matmul(pAB[0:HID, 0:BL], sb["wc1"][:], z0[:])
            c1 = work.tile([HID, BL], f16, tag="c1", name="c1")
            nc.vector.tensor_scalar(
                c1[:], pAB[0:HID, 0:BL], sb["bc1c"][:], 0.0, OP.add, OP.max
            )
            nc.tensor.matmul(pAB[0:NCLS, 128:128 + BL], sb["wc2"][:], c1[:])
            pred = work.tile([NCLS, BL], f32, tag="pred", name="pred")
            nc.vector.tensor_scalar(
                pred[:], pAB[0:NCLS, 128:128 + BL], sb["bc2c"][:], None, OP.add
            )
            nc.sync.dma_start(out_dram[:], pred[:])

    nc.compile()
    return nc


def make_in_maps(inputs):
    shared = _prep_shared(
        inputs["W0"], inputs["b0"], inputs["W_in"], inputs["b_in"],
        inputs["W_h"], inputs["b_h"], inputs["W_out"], inputs["b_out"],
        inputs["Wc1"], inputs["bc1"], inputs["Wc2"], inputs["bc2"],
    )
    bc = np.asarray(inputs["batch_coeffs"], np.float32)
    in_maps = []
    for c in range(NCORES):
        x0t, dxh = _prep_percore(bc[c * BL:(c + 1) * BL])
        in_maps.append({**shared, "x0t": x0t, "dxh": dxh})
    return in_maps


_CACHED = {}


def kernel(**inputs):
    from concourse.bass_utils import run_bass_kernel_spmd

    if "nc" not in _CACHED:
        _CACHED["nc"] = build_nc()
    nc = _CACHED["nc"]
    in_maps = make_in_maps(inputs)
    res = run_bass_kernel_spmd(
        nc, in_maps, core_ids=list(range(NCORES)),
        trace=bool(int(os.environ.get("NCDE_TRACE", "0"))),
    )
    _CACHED["last_result"] = res
    out = np.zeros((B, NCLS), np.float32)
    for c in range(NCORES):
        out[c * BL:(c + 1) * BL, :] = res.results[c]["pred_t"].T
    return out



# revision 29
# speedup vs baseline: 1.0269x; 1.0269x over previous
"""Neural CDE forward on 8 Trainium2 cores — GBS(2,4) extrapolated midpoint.

Replaces the reference's 2x-RK4-substep integration (8 serial vector-field
evals per knot interval) with a Gragg smoothed-midpoint pair (n=2, n=4) and
Richardson extrapolation: T = (4*T4 - T2)/3.  Same order, 7 distinct evals
per interval, packed into 5 PE "ticks" per interval, two of them 128
columns wide:

  t0 (64):  f0 = g(z0, 0)         shared by both sequences
  t1 (128): a1 = g(e1A, .5)   |   b1 = g(e1B, .25)
  t2 (128): a2 = g(e2A, 1)    |   b2 = g(e2B, .5)
  t3 (64):  b3 = g(e3B, .75)
  t4 (64):  b4 = g(e4B, 1)

All state is feature-major [64 hid, batch] f16 so each tick's MLP streams
packed columns through the PE with no transposes.  y accumulates in six
fp32 PSUM bank regions; paired ticks put batch on the partition dim
(p = A|B), solo ticks pack the two h-halves there (p = half*64+b) so their
einsum still uses all 128 lanes at half the per-partition columns.

Scheduling, tuned against hw traces (PE streams f16 at a fixed 0.833ns/col
regardless of continuity; ~110ns+LDW serialization between small matmuls):
 - einsum matmuls are grouped by stationary operand (all woa, then all
   wob) so the PE weight regs reload once per group; PSUM accumulation
   groups stay open concurrently across banks (verified safe on hw);
 - the tanh/mult/reduce drain pipeline chases the einsum region by
   region; k returns to feature-major via ONE block-diagonal permutation
   matmul per 32-h slice (scale constants folded into the tables), so
   state updates start before the drain finishes;
 - the interval-end extrapolation is folded to a single chained op
   (z0' = kB4/6 + [(2/3)(e3B+e4B) + U6]) with the off-chain combines
   (S, V) on the otherwise-idle gpsimd engine;
 - the t0/t1 state-update scales (powers of two, f16-exact) are folded
   into pre-scaled eperm constants so one broadcast scalar_tensor_tensor
   writes both sequence halves at once.
Measured on hw: rel err 8.98e-3 (gate 2e-2), 1018368 ns.
"""

import os
import numpy as np

B, T, IN, HID, HH, NCLS = 512, 16, 41, 64, 150, 4
NL = 3
NCORES = 8
BL = B // NCORES            # 64 per-core batch
INP = IN + 1                # 42: i padded so segments stay 4B-aligned
NIV = T - 1                 # 15 intervals
NTICK = 5
YC = HID * INP              # 2688 y columns
# fp32 PSUM regions: cols (hc*42) <= 512 fp32 per bank; last region small so
# the final tanh/mult/reduce tail is short.
REGIONS = [(0, 12), (12, 12), (24, 8), (32, 12), (44, 12), (56, 8)]
# solo ticks pack the two h-halves into partitions (p = half*64+b) so the
# einsum runs [128, 1344] instead of [64, 2688]; regions are h_local ranges.
# Solo drain is chain-paced (not DVE-bound), so the LAST region is small.
SOLO_REG = [(0, 12), (12, 12), (24, 8)]
# paired-tick h-slices for the pipelined tail; 32-partition aligned
# (matmul out.base_partition must be 0/32/64).
SLICES = [(0, 32, 2), (32, 32, 5)]  # (h0, hc, after-region)
# drain-engine split: the dX mult of EARLY regions goes to gpsimd (Pool,
# otherwise idle); late (chain-critical) regions stay on the faster DVE.
POOL_MULT_MAX = -1
SOLO_POOL_MULT_MAX = -1
# reduces via DVE pool_avg (one InstPool per region, sum/42 folded into dxh)
# — rejected by walrus codegen ("Pool input AP must have 5 dimensions")
USE_POOL_AVG = False
TICK_DX = [
    [0.0, 0.0],
    [0.5, 0.25],
    [1.0, 0.5],
    [0.75, 0.75],
    [1.0, 1.0],
]
TICK_ROWS = [64, 128, 128, 64, 64]
HEAT_MLP = int(os.environ.get("NCDE_HEAT_MLP", "0"))   # heaters before a layer
HEAT_POST = int(os.environ.get("NCDE_HEAT", "0"))      # heaters after W_out
HEAT_COLS = int(os.environ.get("NCDE_HEAT_COLS", "384"))


def _prep_shared(W0, b0, W_in, b_in, W_h, b_h, W_out, b_out, Wc1, bc1, Wc2, bc2):
    f16, f32 = np.float16, np.float32
    whA = np.concatenate([W_h[i][0:128, :] for i in range(NL)], axis=1)
    whB = np.concatenate([W_h[i][128:HH, :] for i in range(NL)], axis=1)
    bias_a = np.stack([b_in[0:128]] + [b_h[i][0:128] for i in range(NL)], axis=1)
    bias_b = np.stack([b_in[128:HH]] + [b_h[i][128:HH] for i in range(NL)], axis=1)
    Rp = np.zeros((HH, HID, INP), np.float32)
    Rp[:, :, :IN] = W_out.reshape(HH, HID, IN)
    W2 = Rp.reshape(HH, YC)
    bo = np.zeros((HID, INP), np.float32)
    bo[:, :IN] = b_out.reshape(HID, IN)
    return {
        "w0": W0.astype(f16),
        "b0c": b0.reshape(HID, 1).astype(f32),
        "wiA": W_in[:, 0:128].astype(f16),
        "wiB": W_in[:, 128:HH].astype(f16),
        "whA": whA.astype(f16),
        "whB": whB.astype(f16),
        "bias_a": bias_a.astype(f32),
        "bias_b": bias_b.astype(f32),
        "woa": W2[0:128].astype(f16),
        "wob": np.vstack([W2[128:HH], bo.reshape(1, YC)]).astype(f16),
        "wc1": Wc1.astype(f16),
        "bc1c": bc1.reshape(HID, 1).astype(f32),
        "wc2": Wc2.astype(f16),
        "bc2c": bc2.reshape(NCLS, 1).astype(f32),
        "id128": np.eye(128, dtype=np.float16),
        "eperm": np.vstack([
            np.hstack([np.eye(64), 0 * np.eye(64)]),
            np.hstack([0 * np.eye(64), np.eye(64)]),
        ]).astype(np.float16),
        "eperm2": np.vstack([
            np.hstack([np.eye(64), 0 * np.eye(64)]),
            np.hstack([0 * np.eye(64), 0.5 * np.eye(64)]),
        ]).astype(np.float16),
        "ep0a": np.hstack([
            0.5 * np.vstack([np.eye(64), 0 * np.eye(64)]),
            0.25 * np.vstack([np.eye(64), 0 * np.eye(64)]),
        ]).astype(np.float16),
        "ep0b": np.hstack([
            0.5 * np.vstack([0 * np.eye(64), np.eye(64)]),
            0.25 * np.vstack([0 * np.eye(64), np.eye(64)]),
        ]).astype(np.float16),
        "hhB_init": np.vstack(
            [np.zeros((HH - 128, BL * 2), np.float32), np.ones((1, BL * 2), np.float32)]
        ).astype(f16),
    }


def _prep_percore(bc_core):
    x0t = bc_core[:, 0, 0, :].T.astype(np.float16)
    c1, c2, c3 = bc_core[:, :, 1, :], bc_core[:, :, 2, :], bc_core[:, :, 3, :]
    dxh = np.zeros((128, NIV * NTICK * INP), np.float32)
    for iv in range(NIV):
        for t in range(NTICK):
            col = (iv * NTICK + t) * INP
            for half, s in enumerate(TICK_DX[t]):
                dX = c1[:, iv] + (2.0 * s) * c2[:, iv] + (3.0 * s * s) * c3[:, iv]
                dxh[half * BL:(half + 1) * BL, col:col + IN] = dX
    if USE_POOL_AVG:
        # k comes out of pool_avg (sum/INP); pre-scale dX so avg(pr) == k
        dxh *= float(INP)
    return x0t, dxh.astype(np.float16)


def build_nc():
    from contextlib import ExitStack

    import concourse.bass as bass
    import concourse.mybir as mybir
    from concourse import bacc, tile

    f16 = mybir.dt.float16
    f32 = mybir.dt.float32
    AF = mybir.ActivationFunctionType
    OP = mybir.AluOpType

    nc = bacc.Bacc("TRN2", target_bir_lowering=False, debug=False)

    dram = {}
    ins_spec = [
        ("x0t", [IN, BL], f16),
        ("dxh", [128, NIV * NTICK * INP], f16),
        ("w0", [IN, HID], f16),
        ("b0c", [HID, 1], f32),
        ("wiA", [HID, 128], f16),
        ("wiB", [HID, HH - 128], f16),
        ("whA", [128, NL * HH], f16),
        ("whB", [HH - 128, NL * HH], f16),
        ("bias_a", [128, 1 + NL], f32),
        ("bias_b", [HH - 128, 1 + NL], f32),
        ("woa", [128, YC], f16),
        ("wob", [HH - 128 + 1, YC], f16),
        ("wc1", [HID, HID], f16),
        ("bc1c", [HID, 1], f32),
        ("wc2", [HID, NCLS], f16),
        ("bc2c", [NCLS, 1], f32),
        ("id128", [128, 128], f16),
        ("eperm", [128, 128], f16),
        ("eperm2", [128, 128], f16),
        ("ep0a", [128, 128], f16),
        ("ep0b", [128, 128], f16),
        ("hhB_init", [HH - 128 + 1, 2 * BL], f16),
    ]
    for name, shape, dt in ins_spec:
        dram[name] = nc.dram_tensor(name, shape, dt, kind="ExternalInput")
    out_dram = nc.dram_tensor("pred_t", [NCLS, BL], f32, kind="ExternalOutput")

    with tile.TileContext(nc) as tc:
        with ExitStack() as ctx:
            const = ctx.enter_context(tc.tile_pool(name="const", bufs=1))
            work = ctx.enter_context(tc.tile_pool(name="work", bufs=2))
            st = ctx.enter_context(tc.tile_pool(name="st", bufs=1))
            ty_pool = ctx.enter_context(tc.tile_pool(name="ty", bufs=2))
            pr_pool = ctx.enter_context(tc.tile_pool(name="pr", bufs=2))
            ps_y = ctx.enter_context(
                tc.tile_pool(name="ps_y", bufs=1, space=bass.MemorySpace.PSUM)
            )
            ps_mlp = ctx.enter_context(
                tc.tile_pool(name="ps_mlp", bufs=1, space=bass.MemorySpace.PSUM)
            )
            ps_k = ctx.enter_context(
                tc.tile_pool(name="ps_k", bufs=1, space=bass.MemorySpace.PSUM)
            )

            sb = {}
            for name, shape, dt in ins_spec:
                tt = const.tile(shape, dt, tag=name, name=name)
                nc.sync.dma_start(tt[:], dram[name][:])
                sb[name] = tt

            yR = [
                ps_y.tile([128, 512], f32, tag=f"yR{r}", name=f"yR{r}")
                for r in range(len(REGIONS))
            ]
            pAB = ps_mlp.tile([128, 256], f32, tag="pAB", name="pAB")
            ktp = ps_k.tile([HID, 2 * BL], f32, tag="ktp", name="ktp")

            z0 = st.tile([HID, BL], f16, tag="z0", name="z0")
            zin1 = st.tile([HID, 2 * BL], f16, tag="zin1", name="zin1")
            zin2 = st.tile([HID, 2 * BL], f16, tag="zin2", name="zin2")
            zin3 = st.tile([HID, BL], f16, tag="zin3", name="zin3")
            zin4 = st.tile([HID, BL], f16, tag="zin4", name="zin4")
            S_t = st.tile([HID, BL], f32, tag="S", name="S_t")
            U6a = st.tile([HID, BL], f32, tag="U6a", name="U6a")
            U6 = st.tile([HID, BL], f32, tag="U6", name="U6")
            V_t = st.tile([HID, BL], f32, tag="V", name="V_t")
            Wt = st.tile([HID, BL], f32, tag="Wt", name="Wt")
            hhB = sb["hhB_init"]

            nc.tensor.matmul(pAB[0:HID, 0:BL], sb["w0"][:], sb["x0t"][:])
            nc.vector.tensor_scalar(z0[:], pAB[0:HID, 0:BL], sb["b0c"][:], None, OP.add)

            def heat(n, rbias=0):
                # heater matmuls: no live data deps (target y banks whose
                # tanh read has already issued), keep the PE p-state warm
                for i in range(n):
                    nc.tensor.matmul(
                        yR[(i + rbias) % len(REGIONS)][0:128, 0:HEAT_COLS],
                        sb["id128"][:],
                        sb["woa"][:, 0:HEAT_COLS],
                        skip_group_check=True,
                    )

            def tick(zin_ap, rows, dxcol, upd, sliced_in, kb_half=False,
                     t0_scale=False):
                """One packed vector-field eval.  After each h-slice's reduce,
                transposes that slice of k and calls upd(h0, hc, kT_slice)."""
                hA = None
                for layer in range(1 + NL):
                    heat(HEAT_MLP, rbias=2 * layer)
                    if layer == 0:
                        if sliced_in:
                            # c-sliced accumulation so layer 0 overlaps the
                            # previous tick's einsum drain.  Groups must be
                            # sequential per bank: all of A, then all of B
                            # (interleaved open groups corrupt PSUM on hw).
                            for g, (h0, hc, _) in enumerate(SLICES):
                                nc.tensor.matmul(
                                    pAB[0:128, 0:rows],
                                    sb["wiA"][h0:h0 + hc, :],
                                    zin_ap[h0:h0 + hc, :],
                                    start=g == 0, stop=g == len(SLICES) - 1,
                                    skip_group_check=True,
                                )
                            for g, (h0, hc, _) in enumerate(SLICES):
                                nc.tensor.matmul(
                                    pAB[0:HH - 128, 128:128 + rows],
                                    sb["wiB"][h0:h0 + hc, :],
                                    zin_ap[h0:h0 + hc, :],
                                    start=g == 0, stop=g == len(SLICES) - 1,
                                    skip_group_check=True,
                                )
                        else:
                            nc.tensor.matmul(
                                pAB[0:128, 0:rows], sb["wiA"][:], zin_ap
                            )
                            nc.tensor.matmul(
                                pAB[0:HH - 128, 128:128 + rows], sb["wiB"][:], zin_ap
                            )
                    else:
                        c0 = (layer - 1) * HH
                        nc.tensor.matmul(
                            pAB[0:128, 0:rows],
                            sb["whA"][:, c0:c0 + 128],
                            hA[:, 0:rows],
                            start=True, stop=False, skip_group_check=True,
                        )
                        nc.tensor.matmul(
                            pAB[0:128, 0:rows],
                            sb["whB"][:, c0:c0 + 128],
                            hhB[0:HH - 128, 0:rows],
                            start=False, stop=True, skip_group_check=True,
                        )
                        nc.tensor.matmul(
                            pAB[0:HH - 128, 128:128 + rows],
                            sb["whA"][:, c0 + 128:c0 + HH],
                            hA[:, 0:rows],
                            start=True, stop=False, skip_group_check=True,
                        )
                        nc.tensor.matmul(
                            pAB[0:HH - 128, 128:128 + rows],
                            sb["whB"][:, c0 + 128:c0 + HH],
                            hhB[0:HH - 128, 0:rows],
                            start=False, stop=True, skip_group_check=True,
                        )
                    nhA = work.tile([128, 2 * BL], f16, tag="hA", name="nhA")
                    ba = sb["bias_a"][:, layer:layer + 1]
                    bb = sb["bias_b"][:, layer:layer + 1]
                    nc.scalar.activation(
                        nhA[:, 0:rows], pAB[0:128, 0:rows], AF.Relu, bias=ba
                    )
                    # B-half relu on DVE (gpsimd cannot read PSUM)
                    nc.vector.tensor_scalar(
                        hhB[0:HH - 128, 0:rows],
                        pAB[0:HH - 128, 128:128 + rows],
                        bb, 0.0, OP.add, OP.max,
                    )
                    hA = nhA

                paired = rows == 128
                regs = REGIONS if paired else SOLO_REG
                # einsum grouped by stationary operand (hA for woa, hhB for
                # wob) so the PE's weight regs reload only once per group —
                # kills the ~140ns LDW transition bubble between regions.
                if paired:
                    for r, (h0, hc) in enumerate(regs):
                        nc.tensor.matmul(
                            yR[r][0:128, 0:hc * INP],
                            hA[:, 0:rows],
                            sb["woa"][:, h0 * INP:(h0 + hc) * INP],
                            start=True, stop=False, skip_group_check=True,
                        )
                    for r, (h0, hc) in enumerate(regs):
                        nc.tensor.matmul(
                            yR[r][0:128, 0:hc * INP],
                            hhB[:, 0:rows],
                            sb["wob"][:, h0 * INP:(h0 + hc) * INP],
                            start=False, stop=True, skip_group_check=True,
                        )
                else:
                    for r, (h0, hc) in enumerate(regs):
                        for half in range(2):
                            c0 = (half * 32 + h0) * INP
                            nc.tensor.matmul(
                                yR[r][half * 64:half * 64 + 64, 0:hc * INP],
                                hA[:, 0:rows],
                                sb["woa"][:, c0:c0 + hc * INP],
                                start=True, stop=False, skip_group_check=True,
                            )
                    for r, (h0, hc) in enumerate(regs):
                        for half in range(2):
                            c0 = (half * 32 + h0) * INP
                            nc.tensor.matmul(
                                yR[r][half * 64:half * 64 + 64, 0:hc * INP],
                                hhB[:, 0:rows],
                                sb["wob"][:, c0:c0 + hc * INP],
                                start=False, stop=True, skip_group_check=True,
                            )
                # per-slice k tiles (reduced at free offset 0)
                k_sl = [
                    work.tile([128, 48], f16, tag=f"k_t{i}", name="k_sl")
                    for i in range(2)
                ]
                slice_i = 0
                for r, (h0, hc) in enumerate(regs):
                    ty = ty_pool.tile([128, 1024], f16, tag=f"ty{r % 2}", name="ty")
                    nc.scalar.activation(
                        ty[0:128, 0:hc * INP], yR[r][0:128, 0:hc * INP], AF.Tanh
                    )
                    pr = pr_pool.tile([128, 1024], f16, tag=f"pr{r % 2}", name="pr")
                    dxv = (
                        sb["dxh"][0:128, dxcol:dxcol + INP]
                        .unsqueeze(1)
                        .broadcast_to((128, hc, INP))
                    )
                    tyv = ty[0:128, 0:hc * INP].rearrange("p (h i) -> p h i", i=INP)
                    prv = pr[0:128, 0:hc * INP].rearrange("p (h i) -> p h i", i=INP)
                    pm = POOL_MULT_MAX if paired else SOLO_POOL_MULT_MAX
                    if r <= pm:
                        # early regions' mult on the otherwise-idle gpsimd
                        nc.gpsimd.tensor_tensor(prv, tyv, dxv, OP.mult)
                    else:
                        nc.vector.tensor_tensor(prv, tyv, dxv, OP.mult)
                    ksl = k_sl[slice_i] if paired else k_sl[0]
                    koff = SLICES[slice_i][0] if paired else 0
                    if USE_POOL_AVG:
                        nc.vector.pool(
                            ksl[0:128, h0 - koff:h0 - koff + hc],
                            prv, mybir.PoolFunctionType.avg,
                        )
                    else:
                        with nc.allow_low_precision(
                            reason="f16 rounding of final k"
                        ):
                            nc.vector.tensor_reduce(
                                ksl[0:128, h0 - koff:h0 - koff + hc],
                                prv, mybir.AxisListType.X, OP.add,
                            )
                    if paired and SLICES[slice_i][2] == r:
                        # k^T slice via ONE eperm matmul (block-diagonal
                        # permutation): ktp[s0:s0+32, 0:64]=kA, [..,64:128]=kB
                        # (eperm2's cols 0:64 equal eperm's, so one 128-col
                        # rhs covers both halves)
                        s0, sc, _ = SLICES[slice_i]
                        nc.tensor.matmul(
                            ktp[s0:s0 + sc, 0:2 * BL],
                            k_sl[slice_i][:, 0:sc],
                            (sb["eperm2"] if kb_half else sb["eperm"])
                            [:, 0:128],
                        )
                        upd(s0, sc)
                        slice_i += 1
                if not paired:
                    # k_sl[0][p=(half,b), h_local] -> ktp[h, b] f32; for t0
                    # the double-width scaled eperms emit [0.5*k | 0.25*k]
                    if t0_scale:
                        nc.tensor.matmul(
                            ktp[0:32, 0:2 * BL], k_sl[0][:, 0:32],
                            sb["ep0a"][:, 0:128],
                        )
                        nc.tensor.matmul(
                            ktp[32:64, 0:2 * BL], k_sl[0][:, 0:32],
                            sb["ep0b"][:, 0:128],
                        )
                    else:
                        nc.tensor.matmul(
                            ktp[0:32, 0:BL], k_sl[0][:, 0:32], sb["eperm"][:, 0:64]
                        )
                        nc.tensor.matmul(
                            ktp[32:64, 0:BL], k_sl[0][:, 0:32],
                            sb["eperm"][:, 64:128],
                        )
                    upd(0, 64)

            STT = nc.vector.scalar_tensor_tensor

            for iv in range(NIV):
                base = iv * NTICK * INP
                kA = lambda s0, sc: ktp[s0:s0 + sc, 0:BL]
                kB = lambda s0, sc: ktp[s0:s0 + sc, BL:2 * BL]

                def upd0(s0, sc):
                    # ktp holds [0.5*k | 0.25*k]; one broadcast STT writes
                    # both e1A and e1B
                    sl = slice(s0, s0 + sc)
                    o3 = zin1[sl, 0:2 * BL].rearrange("p (g b) -> p g b", b=BL)
                    k3 = ktp[sl, 0:2 * BL].rearrange("p (g b) -> p g b", b=BL)
                    z3 = z0[sl, :].unsqueeze(1).broadcast_to((sc, 2, BL))
                    STT(o3, k3, 1.0, z3, OP.mult, OP.add)

                tick(z0[:], 64, base, upd0, sliced_in=False, t0_scale=True)

                def upd1(s0, sc):
                    # ktp holds [kA | 0.5*kB] (eperm2); t2's layer-0 is
                    # whole-tile-gated, so one full-range broadcast STT after
                    # the final slice covers everything
                    if s0 == 0:
                        return
                    o3 = zin2[0:HID, 0:2 * BL].rearrange("p (g b) -> p g b", b=BL)
                    k3 = ktp[0:HID, 0:2 * BL].rearrange("p (g b) -> p g b", b=BL)
                    z3 = z0[0:HID, :].unsqueeze(1).broadcast_to((HID, 2, BL))
                    STT(o3, k3, 1.0, z3, OP.mult, OP.add)

                tick(zin1[:], 128, base + 1 * INP, upd1, sliced_in=False,
                     kb_half=True)

                def upd2(s0, sc):
                    if s0 == 0:
                        return
                    STT(zin3[0:HID, :], kB(0, HID), 0.5,
                        zin1[0:HID, BL:2 * BL], OP.mult, OP.add)

                tick(zin2[:], 128, base + 2 * INP, upd2, sliced_in=False)
                # off-chain A-branch combine: U6 = -(e1A+e2A+.5kA2)/6
                # (S_t/U6 on gpsimd; U6a reads ktp=PSUM so it stays on DVE)
                nc.gpsimd.tensor_tensor(S_t[:], zin1[:, 0:BL], zin2[:, 0:BL], OP.add)
                STT(U6a[:], ktp[0:HID, 0:BL], 0.5, S_t[:], OP.mult, OP.add)
                nc.vector.tensor_scalar(U6[:], U6a[:], -1.0 / 6.0, None, OP.mult)

                def upd3(s0, sc):
                    sl = slice(s0, s0 + sc)
                    STT(zin4[sl, :], kA(s0, sc), 0.5, zin2[sl, BL:2 * BL], OP.mult, OP.add)

                tick(zin3[:], 64, base + 3 * INP, upd3, sliced_in=False)
                # z0' = (4*T4 - T2)/3 = kB4/6 + VU,  VU = (2/3)(e3B+e4B) + U6
                nc.gpsimd.tensor_tensor(V_t[:], zin3[:], zin4[:], OP.add)
                STT(Wt[:], V_t[:], 2.0 / 3.0, U6[:], OP.mult, OP.add)

                def upd4(s0, sc):
                    sl = slice(s0, s0 + sc)
                    STT(z0[sl, :], kA(s0, sc), 1.0 / 6.0, Wt[sl, :], OP.mult, OP.add)

                tick(zin4[:], 64, base + 4 * INP, upd4, sliced_in=False)

            nc.tensor.matmul(pAB[0:HID, 0:BL], sb["wc1"][:], z0[:])
            c1 = work.tile([HID, BL], f16, tag="c1", name="c1")
            nc.vector.tensor_scalar(
                c1[:], pAB[0:HID, 0:BL], sb["bc1c"][:], 0.0, OP.add, OP.max
            )
            nc.tensor.matmul(pAB[0:NCLS, 128:128 + BL], sb["wc2"][:], c1[:])
            pred = work.tile([NCLS, BL], f32, tag="pred", name="pred")
            nc.vector.tensor_scalar(
                pred[:], pAB[0:NCLS, 128:128 + BL], sb["bc2c"][:], None, OP.add
            )
            nc.sync.dma_start(out_dram[:], pred[:])

    nc.compile()
    return nc


def make_in_maps(inputs):
    shared = _prep_shared(
        inputs["W0"], inputs["b0"], inputs["W_in"], inputs["b_in"],
        inputs["W_h"], inputs["b_h"], inputs["W_out"], inputs["b_out"],
        inputs["Wc1"], inputs["bc1"], inputs["Wc2"], inputs["bc2"],
    )
    bc = np.asarray(inputs["batch_coeffs"], np.float32)
    in_maps = []
    for c in range(NCORES):
        x0t, dxh = _prep_percore(bc[c * BL:(c + 1) * BL])
        in_maps.append({**shared, "x0t": x0t, "dxh": dxh})
    return in_maps


_CACHED = {}


def kernel(**inputs):
    from concourse.bass_utils import run_bass_kernel_spmd

    if "nc" not in _CACHED:
        _CACHED["nc"] = build_nc()
    nc = _CACHED["nc"]
    in_maps = make_in_maps(inputs)
    res = run_bass_kernel_spmd(
        nc, in_maps, core_ids=list(range(NCORES)),
        trace=bool(int(os.environ.get("NCDE_TRACE", "0"))),
    )
    _CACHED["last_result"] = res
    out = np.zeros((B, NCLS), np.float32)
    for c in range(NCORES):
        out[c * BL:(c + 1) * BL, :] = res.results[c]["pred_t"].T
    return out



# revision 30
# speedup vs baseline: 1.0711x; 1.0430x over previous
"""Neural CDE forward on 8 Trainium2 cores — GBS(2,4) extrapolated midpoint.

Replaces the reference's 2x-RK4-substep integration (8 serial vector-field
evals per knot interval) with a Gragg smoothed-midpoint pair (n=2, n=4) and
Richardson extrapolation: T = (4*T4 - T2)/3.  Same order, 7 distinct evals
per interval, packed into 5 PE "ticks" per interval, two of them 128
columns wide:

  t0 (64):  f0 = g(z0, 0)         shared by both sequences
  t1 (128): a1 = g(e1A, .5)   |   b1 = g(e1B, .25)
  t2 (128): a2 = g(e2A, 1)    |   b2 = g(e2B, .5)
  t3 (64):  b3 = g(e3B, .75)
  t4 (64):  b4 = g(e4B, 1)

All state is feature-major [64 hid, batch] f16 so each tick's MLP streams
packed columns through the PE with no transposes.  y accumulates in six
fp32 PSUM bank regions; paired ticks put batch on the partition dim
(p = A|B), solo ticks pack the two h-halves there (p = half*64+b) so their
einsum still uses all 128 lanes at half the per-partition columns.

Scheduling, tuned against hw traces (PE streams f16 at a fixed 0.833ns/col
regardless of continuity; ~110ns+LDW serialization between small matmuls):
 - einsum matmuls are grouped by stationary operand (all woa, then all
   wob) so the PE weight regs reload once per group; PSUM accumulation
   groups stay open concurrently across banks (verified safe on hw);
 - the tanh/mult/reduce drain pipeline chases the einsum region by
   region; k returns to feature-major via ONE block-diagonal permutation
   matmul per 32-h slice (scale constants folded into the tables), so
   state updates start before the drain finishes;
 - the interval-end extrapolation is folded to a single chained op
   (z0' = kB4/6 + [(2/3)(e3B+e4B) + U6]) with the off-chain combines
   (S, V) on the otherwise-idle gpsimd engine;
 - the t0/t1 state-update scales (powers of two, f16-exact) are folded
   into pre-scaled eperm constants so one broadcast scalar_tensor_tensor
   writes both sequence halves at once.
Measured on hw: rel err 8.98e-3 (gate 2e-2), 1018368 ns.
"""

import os
import numpy as np

B, T, IN, HID, HH, NCLS = 512, 16, 41, 64, 150, 4
NL = 3
NCORES = 8
BL = B // NCORES            # 64 per-core batch
INP = IN + 1                # 42: i padded so segments stay 4B-aligned
NIV = T - 1                 # 15 intervals
NTICK = 5
YC = HID * INP              # 2688 y columns
# fp32 PSUM regions: cols (hc*42) <= 512 fp32 per bank; last region small so
# the final tanh/mult/reduce tail is short.
REGIONS = [(0, 12), (12, 12), (24, 8), (32, 12), (44, 12), (56, 8)]
# solo ticks pack the two h-halves into partitions (p = half*64+b) so the
# einsum runs [128, 1344] instead of [64, 2688]; regions are h_local ranges.
# Solo drain is chain-paced (not DVE-bound), so the LAST region is small.
SOLO_REG = [(0, 12), (12, 12), (24, 8)]
# paired-tick h-slices for the pipelined tail; 32-partition aligned
# (matmul out.base_partition must be 0/32/64).
SLICES = [(0, 32, 2), (32, 32, 5)]  # (h0, hc, after-region)
# drain-engine split: the dX mult of EARLY regions goes to gpsimd (Pool,
# otherwise idle); late (chain-critical) regions stay on the faster DVE.
POOL_MULT_MAX = -1
SOLO_POOL_MULT_MAX = -1
# reduces via DVE pool_avg (one InstPool per region, sum/42 folded into dxh)
# — rejected by walrus codegen ("Pool input AP must have 5 dimensions")
USE_POOL_AVG = False
TICK_DX = [
    [0.0, 0.0],
    [0.5, 0.25],
    [1.0, 0.5],
    [0.75, 0.75],
    [1.0, 1.0],
]
TICK_ROWS = [64, 128, 128, 64, 64]
HEAT_MLP = int(os.environ.get("NCDE_HEAT_MLP", "0"))   # heaters before a layer
HEAT_POST = int(os.environ.get("NCDE_HEAT", "0"))      # heaters after W_out
HEAT_COLS = int(os.environ.get("NCDE_HEAT_COLS", "384"))


def _prep_shared(W0, b0, W_in, b_in, W_h, b_h, W_out, b_out, Wc1, bc1, Wc2, bc2):
    f16, f32 = np.float16, np.float32
    whA = np.concatenate([W_h[i][0:128, :] for i in range(NL)], axis=1)
    whB = np.concatenate([W_h[i][128:HH, :] for i in range(NL)], axis=1)
    bias_a = np.stack([b_in[0:128]] + [b_h[i][0:128] for i in range(NL)], axis=1)
    bias_b = np.stack([b_in[128:HH]] + [b_h[i][128:HH] for i in range(NL)], axis=1)
    Rp = np.zeros((HH, HID, INP), np.float32)
    Rp[:, :, :IN] = W_out.reshape(HH, HID, IN)
    W2 = Rp.reshape(HH, YC)
    bo = np.zeros((HID, INP), np.float32)
    bo[:, :IN] = b_out.reshape(HID, IN)
    return {
        "w0": W0.astype(f16),
        "b0c": b0.reshape(HID, 1).astype(f32),
        "wiA": W_in[:, 0:128].astype(f16),
        "wiB": W_in[:, 128:HH].astype(f16),
        "whA": whA.astype(f16),
        "whB": whB.astype(f16),
        "bias_a": bias_a.astype(f32),
        "bias_b": bias_b.astype(f32),
        "woa": W2[0:128].astype(f16),
        "wob": np.vstack([W2[128:HH], bo.reshape(1, YC)]).astype(f16),
        "wc1": Wc1.astype(f16),
        "bc1c": bc1.reshape(HID, 1).astype(f32),
        "wc2": Wc2.astype(f16),
        "bc2c": bc2.reshape(NCLS, 1).astype(f32),
        "id128": np.eye(128, dtype=np.float16),
        "eperm": np.vstack([
            np.hstack([np.eye(64), 0 * np.eye(64)]),
            np.hstack([0 * np.eye(64), np.eye(64)]),
        ]).astype(np.float16),
        "eperm2": np.vstack([
            np.hstack([np.eye(64), 0 * np.eye(64)]),
            np.hstack([0 * np.eye(64), 0.5 * np.eye(64)]),
        ]).astype(np.float16),
        "ep0a": np.hstack([
            0.5 * np.vstack([np.eye(64), 0 * np.eye(64)]),
            0.25 * np.vstack([np.eye(64), 0 * np.eye(64)]),
        ]).astype(np.float16),
        "ep0b": np.hstack([
            0.5 * np.vstack([0 * np.eye(64), np.eye(64)]),
            0.25 * np.vstack([0 * np.eye(64), np.eye(64)]),
        ]).astype(np.float16),
        "hhB_init": np.vstack(
            [np.zeros((HH - 128, BL * 2), np.float32), np.ones((1, BL * 2), np.float32)]
        ).astype(f16),
    }


def _prep_percore(bc_core):
    x0t = bc_core[:, 0, 0, :].T.astype(np.float16)
    c1, c2, c3 = bc_core[:, :, 1, :], bc_core[:, :, 2, :], bc_core[:, :, 3, :]
    dxh = np.zeros((128, NIV * NTICK * INP), np.float32)
    for iv in range(NIV):
        for t in range(NTICK):
            col = (iv * NTICK + t) * INP
            for half, s in enumerate(TICK_DX[t]):
                dX = c1[:, iv] + (2.0 * s) * c2[:, iv] + (3.0 * s * s) * c3[:, iv]
                dxh[half * BL:(half + 1) * BL, col:col + IN] = dX
    if USE_POOL_AVG:
        # k comes out of pool_avg (sum/INP); pre-scale dX so avg(pr) == k
        dxh *= float(INP)
    return x0t, dxh.astype(np.float16)


def build_nc():
    from contextlib import ExitStack

    import concourse.bass as bass
    import concourse.mybir as mybir
    from concourse import bacc, tile

    f16 = mybir.dt.float16
    f32 = mybir.dt.float32
    AF = mybir.ActivationFunctionType
    OP = mybir.AluOpType

    nc = bacc.Bacc("TRN2", target_bir_lowering=False, debug=False)

    dram = {}
    ins_spec = [
        ("x0t", [IN, BL], f16),
        ("dxh", [128, NIV * NTICK * INP], f16),
        ("w0", [IN, HID], f16),
        ("b0c", [HID, 1], f32),
        ("wiA", [HID, 128], f16),
        ("wiB", [HID, HH - 128], f16),
        ("whA", [128, NL * HH], f16),
        ("whB", [HH - 128, NL * HH], f16),
        ("bias_a", [128, 1 + NL], f32),
        ("bias_b", [HH - 128, 1 + NL], f32),
        ("woa", [128, YC], f16),
        ("wob", [HH - 128 + 1, YC], f16),
        ("wc1", [HID, HID], f16),
        ("bc1c", [HID, 1], f32),
        ("wc2", [HID, NCLS], f16),
        ("bc2c", [NCLS, 1], f32),
        ("id128", [128, 128], f16),
        ("eperm", [128, 128], f16),
        ("eperm2", [128, 128], f16),
        ("ep0a", [128, 128], f16),
        ("ep0b", [128, 128], f16),
        ("hhB_init", [HH - 128 + 1, 2 * BL], f16),
    ]
    for name, shape, dt in ins_spec:
        dram[name] = nc.dram_tensor(name, shape, dt, kind="ExternalInput")
    out_dram = nc.dram_tensor("pred_t", [NCLS, BL], f32, kind="ExternalOutput")

    with tile.TileContext(nc) as tc:
        with ExitStack() as ctx:
            const = ctx.enter_context(tc.tile_pool(name="const", bufs=1))
            work = ctx.enter_context(tc.tile_pool(name="work", bufs=2))
            st = ctx.enter_context(tc.tile_pool(name="st", bufs=1))
            ty_pool = ctx.enter_context(tc.tile_pool(name="ty", bufs=2))
            pr_pool = ctx.enter_context(tc.tile_pool(name="pr", bufs=2))
            ps_y = ctx.enter_context(
                tc.tile_pool(name="ps_y", bufs=1, space=bass.MemorySpace.PSUM)
            )
            ps_mlp = ctx.enter_context(
                tc.tile_pool(name="ps_mlp", bufs=1, space=bass.MemorySpace.PSUM)
            )
            ps_k = ctx.enter_context(
                tc.tile_pool(name="ps_k", bufs=1, space=bass.MemorySpace.PSUM)
            )

            sb = {}
            for name, shape, dt in ins_spec:
                tt = const.tile(shape, dt, tag=name, name=name)
                nc.sync.dma_start(tt[:], dram[name][:])
                sb[name] = tt

            yR = [
                ps_y.tile([128, 512], f32, tag=f"yR{r}", name=f"yR{r}")
                for r in range(len(REGIONS))
            ]
            pAB = ps_mlp.tile([128, 256], f32, tag="pAB", name="pAB")
            ktp = ps_k.tile([HID, 2 * BL], f32, tag="ktp", name="ktp")

            z0 = st.tile([HID, BL], f16, tag="z0", name="z0")
            zin1 = st.tile([HID, 2 * BL], f16, tag="zin1", name="zin1")
            zin2 = st.tile([HID, 2 * BL], f16, tag="zin2", name="zin2")
            zin3 = st.tile([HID, BL], f16, tag="zin3", name="zin3")
            zin4 = st.tile([HID, BL], f16, tag="zin4", name="zin4")
            S_t = st.tile([HID, BL], f32, tag="S", name="S_t")
            U6a = st.tile([HID, BL], f32, tag="U6a", name="U6a")
            U6 = st.tile([HID, BL], f32, tag="U6", name="U6")
            V_t = st.tile([HID, BL], f32, tag="V", name="V_t")
            Wt = st.tile([HID, BL], f32, tag="Wt", name="Wt")
            hhB = sb["hhB_init"]

            nc.tensor.matmul(pAB[0:HID, 0:BL], sb["w0"][:], sb["x0t"][:])
            nc.vector.tensor_scalar(z0[:], pAB[0:HID, 0:BL], sb["b0c"][:], None, OP.add)

            def heat(n, rbias=0):
                # heater matmuls: no live data deps (target y banks whose
                # tanh read has already issued), keep the PE p-state warm
                for i in range(n):
                    nc.tensor.matmul(
                        yR[(i + rbias) % len(REGIONS)][0:128, 0:HEAT_COLS],
                        sb["id128"][:],
                        sb["woa"][:, 0:HEAT_COLS],
                        skip_group_check=True,
                    )

            def tick(zin_ap, rows, dxcol, upd, sliced_in, kb_half=False,
                     t0_scale=False):
                """One packed vector-field eval.  After each h-slice's reduce,
                transposes that slice of k and calls upd(h0, hc, kT_slice)."""
                hA = None
                for layer in range(1 + NL):
                    heat(HEAT_MLP, rbias=2 * layer)
                    if layer == 0:
                        if sliced_in:
                            # c-sliced accumulation so layer 0 overlaps the
                            # previous tick's einsum drain.  Groups must be
                            # sequential per bank: all of A, then all of B
                            # (interleaved open groups corrupt PSUM on hw).
                            for g, (h0, hc, _) in enumerate(SLICES):
                                nc.tensor.matmul(
                                    pAB[0:128, 0:rows],
                                    sb["wiA"][h0:h0 + hc, :],
                                    zin_ap[h0:h0 + hc, :],
                                    start=g == 0, stop=g == len(SLICES) - 1,
                                    skip_group_check=True,
                                )
                            for g, (h0, hc, _) in enumerate(SLICES):
                                nc.tensor.matmul(
                                    pAB[0:HH - 128, 128:128 + rows],
                                    sb["wiB"][h0:h0 + hc, :],
                                    zin_ap[h0:h0 + hc, :],
                                    start=g == 0, stop=g == len(SLICES) - 1,
                                    skip_group_check=True,
                                )
                        else:
                            nc.tensor.matmul(
                                pAB[0:128, 0:rows], sb["wiA"][:], zin_ap
                            )
                            nc.tensor.matmul(
                                pAB[0:HH - 128, 128:128 + rows], sb["wiB"][:], zin_ap
                            )
                    else:
                        c0 = (layer - 1) * HH
                        nc.tensor.matmul(
                            pAB[0:128, 0:rows],
                            sb["whA"][:, c0:c0 + 128],
                            hA[:, 0:rows],
                            start=True, stop=False, skip_group_check=True,
                        )
                        nc.tensor.matmul(
                            pAB[0:128, 0:rows],
                            sb["whB"][:, c0:c0 + 128],
                            hhB[0:HH - 128, 0:rows],
                            start=False, stop=True, skip_group_check=True,
                        )
                        nc.tensor.matmul(
                            pAB[0:HH - 128, 128:128 + rows],
                            sb["whA"][:, c0 + 128:c0 + HH],
                            hA[:, 0:rows],
                            start=True, stop=False, skip_group_check=True,
                        )
                        nc.tensor.matmul(
                            pAB[0:HH - 128, 128:128 + rows],
                            sb["whB"][:, c0 + 128:c0 + HH],
                            hhB[0:HH - 128, 0:rows],
                            start=False, stop=True, skip_group_check=True,
                        )
                    nhA = work.tile([128, 2 * BL], f16, tag="hA", name="nhA")
                    ba = sb["bias_a"][:, layer:layer + 1]
                    bb = sb["bias_b"][:, layer:layer + 1]
                    nc.scalar.activation(
                        nhA[:, 0:rows], pAB[0:128, 0:rows], AF.Relu, bias=ba
                    )
                    # B-half relu on DVE (gpsimd cannot read PSUM)
                    nc.vector.tensor_scalar(
                        hhB[0:HH - 128, 0:rows],
                        pAB[0:HH - 128, 128:128 + rows],
                        bb, 0.0, OP.add, OP.max,
                    )
                    hA = nhA

                paired = rows == 128
                regs = REGIONS if paired else SOLO_REG
                # einsum emitted per-region ([woa-r, wob-r] pairs): the
                # drain pipeline (tanh/mult/reduce) starts chasing at mm #2
                # instead of mm #7 — full lhsT-grouping starved the drains
                # and piled a ~2.5us DVE backlog onto every tick boundary
                # (measured), while saving almost nothing in mm busy time.
                if paired:
                    for r, (h0, hc) in enumerate(regs):
                        nc.tensor.matmul(
                            yR[r][0:128, 0:hc * INP],
                            hA[:, 0:rows],
                            sb["woa"][:, h0 * INP:(h0 + hc) * INP],
                            start=True, stop=False, skip_group_check=True,
                        )
                        nc.tensor.matmul(
                            yR[r][0:128, 0:hc * INP],
                            hhB[:, 0:rows],
                            sb["wob"][:, h0 * INP:(h0 + hc) * INP],
                            start=False, stop=True, skip_group_check=True,
                        )
                else:
                    for r, (h0, hc) in enumerate(regs):
                        for half in range(2):
                            c0 = (half * 32 + h0) * INP
                            nc.tensor.matmul(
                                yR[r][half * 64:half * 64 + 64, 0:hc * INP],
                                hA[:, 0:rows],
                                sb["woa"][:, c0:c0 + hc * INP],
                                start=True, stop=False, skip_group_check=True,
                            )
                        for half in range(2):
                            c0 = (half * 32 + h0) * INP
                            nc.tensor.matmul(
                                yR[r][half * 64:half * 64 + 64, 0:hc * INP],
                                hhB[:, 0:rows],
                                sb["wob"][:, c0:c0 + hc * INP],
                                start=False, stop=True, skip_group_check=True,
                            )
                # per-slice k tiles (reduced at free offset 0)
                k_sl = [
                    work.tile([128, 48], f16, tag=f"k_t{i}", name="k_sl")
                    for i in range(2)
                ]
                slice_i = 0
                for r, (h0, hc) in enumerate(regs):
                    ty = ty_pool.tile([128, 1024], f16, tag=f"ty{r % 2}", name="ty")
                    nc.scalar.activation(
                        ty[0:128, 0:hc * INP], yR[r][0:128, 0:hc * INP], AF.Tanh
                    )
                    pr = pr_pool.tile([128, 1024], f16, tag=f"pr{r % 2}", name="pr")
                    dxv = (
                        sb["dxh"][0:128, dxcol:dxcol + INP]
                        .unsqueeze(1)
                        .broadcast_to((128, hc, INP))
                    )
                    tyv = ty[0:128, 0:hc * INP].rearrange("p (h i) -> p h i", i=INP)
                    prv = pr[0:128, 0:hc * INP].rearrange("p (h i) -> p h i", i=INP)
                    pm = POOL_MULT_MAX if paired else SOLO_POOL_MULT_MAX
                    if r <= pm:
                        # early regions' mult on the otherwise-idle gpsimd
                        nc.gpsimd.tensor_tensor(prv, tyv, dxv, OP.mult)
                    else:
                        nc.vector.tensor_tensor(prv, tyv, dxv, OP.mult)
                    ksl = k_sl[slice_i] if paired else k_sl[0]
                    koff = SLICES[slice_i][0] if paired else 0
                    if USE_POOL_AVG:
                        nc.vector.pool(
                            ksl[0:128, h0 - koff:h0 - koff + hc],
                            prv, mybir.PoolFunctionType.avg,
                        )
                    else:
                        with nc.allow_low_precision(
                            reason="f16 rounding of final k"
                        ):
                            nc.vector.tensor_reduce(
                                ksl[0:128, h0 - koff:h0 - koff + hc],
                                prv, mybir.AxisListType.X, OP.add,
                            )
                    if paired and SLICES[slice_i][2] == r:
                        # k^T slice via ONE eperm matmul (block-diagonal
                        # permutation): ktp[s0:s0+32, 0:64]=kA, [..,64:128]=kB
                        # (eperm2's cols 0:64 equal eperm's, so one 128-col
                        # rhs covers both halves)
                        s0, sc, _ = SLICES[slice_i]
                        nc.tensor.matmul(
                            ktp[s0:s0 + sc, 0:2 * BL],
                            k_sl[slice_i][:, 0:sc],
                            (sb["eperm2"] if kb_half else sb["eperm"])
                            [:, 0:128],
                        )
                        upd(s0, sc)
                        slice_i += 1
                if not paired:
                    # k_sl[0][p=(half,b), h_local] -> ktp[h, b] f32; for t0
                    # the double-width scaled eperms emit [0.5*k | 0.25*k]
                    if t0_scale:
                        nc.tensor.matmul(
                            ktp[0:32, 0:2 * BL], k_sl[0][:, 0:32],
                            sb["ep0a"][:, 0:128],
                        )
                        nc.tensor.matmul(
                            ktp[32:64, 0:2 * BL], k_sl[0][:, 0:32],
                            sb["ep0b"][:, 0:128],
                        )
                    else:
                        nc.tensor.matmul(
                            ktp[0:32, 0:BL], k_sl[0][:, 0:32], sb["eperm"][:, 0:64]
                        )
                        nc.tensor.matmul(
                            ktp[32:64, 0:BL], k_sl[0][:, 0:32],
                            sb["eperm"][:, 64:128],
                        )
                    upd(0, 64)

            STT = nc.vector.scalar_tensor_tensor

            for iv in range(NIV):
                base = iv * NTICK * INP
                kA = lambda s0, sc: ktp[s0:s0 + sc, 0:BL]
                kB = lambda s0, sc: ktp[s0:s0 + sc, BL:2 * BL]

                def upd0(s0, sc):
                    # ktp holds [0.5*k | 0.25*k]; one broadcast STT writes
                    # both e1A and e1B
                    sl = slice(s0, s0 + sc)
                    o3 = zin1[sl, 0:2 * BL].rearrange("p (g b) -> p g b", b=BL)
                    k3 = ktp[sl, 0:2 * BL].rearrange("p (g b) -> p g b", b=BL)
                    z3 = z0[sl, :].unsqueeze(1).broadcast_to((sc, 2, BL))
                    STT(o3, k3, 1.0, z3, OP.mult, OP.add)

                tick(z0[:], 64, base, upd0, sliced_in=False, t0_scale=True)

                def upd1(s0, sc):
                    # ktp holds [kA | 0.5*kB] (eperm2); t2's layer-0 is
                    # whole-tile-gated, so one full-range broadcast STT after
                    # the final slice covers everything
                    if s0 == 0:
                        return
                    o3 = zin2[0:HID, 0:2 * BL].rearrange("p (g b) -> p g b", b=BL)
                    k3 = ktp[0:HID, 0:2 * BL].rearrange("p (g b) -> p g b", b=BL)
                    z3 = z0[0:HID, :].unsqueeze(1).broadcast_to((HID, 2, BL))
                    STT(o3, k3, 1.0, z3, OP.mult, OP.add)

                tick(zin1[:], 128, base + 1 * INP, upd1, sliced_in=False,
                     kb_half=True)

                def upd2(s0, sc):
                    if s0 == 0:
                        return
                    STT(zin3[0:HID, :], kB(0, HID), 0.5,
                        zin1[0:HID, BL:2 * BL], OP.mult, OP.add)

                tick(zin2[:], 128, base + 2 * INP, upd2, sliced_in=False)
                # off-chain A-branch combine: U6 = -(e1A+e2A+.5kA2)/6
                # (S_t/U6 on gpsimd; U6a reads ktp=PSUM so it stays on DVE)
                nc.gpsimd.tensor_tensor(S_t[:], zin1[:, 0:BL], zin2[:, 0:BL], OP.add)
                STT(U6a[:], ktp[0:HID, 0:BL], 0.5, S_t[:], OP.mult, OP.add)
                nc.vector.tensor_scalar(U6[:], U6a[:], -1.0 / 6.0, None, OP.mult)

                def upd3(s0, sc):
                    sl = slice(s0, s0 + sc)
                    STT(zin4[sl, :], kA(s0, sc), 0.5, zin2[sl, BL:2 * BL], OP.mult, OP.add)

                tick(zin3[:], 64, base + 3 * INP, upd3, sliced_in=False)
                # z0' = (4*T4 - T2)/3 = kB4/6 + VU,  VU = (2/3)(e3B+e4B) + U6
                nc.gpsimd.tensor_tensor(V_t[:], zin3[:], zin4[:], OP.add)
                STT(Wt[:], V_t[:], 2.0 / 3.0, U6[:], OP.mult, OP.add)

                def upd4(s0, sc):
                    sl = slice(s0, s0 + sc)
                    STT(z0[sl, :], kA(s0, sc), 1.0 / 6.0, Wt[sl, :], OP.mult, OP.add)

                tick(zin4[:], 64, base + 4 * INP, upd4, sliced_in=False)

            nc.tensor.matmul(pAB[0:HID, 0:BL], sb["wc1"][:], z0[:])
            c1 = work.tile([HID, BL], f16, tag="c1", name="c1")
            nc.vector.tensor_scalar(
                c1[:], pAB[0:HID, 0:BL], sb["bc1c"][:], 0.0, OP.add, OP.max
            )
            nc.tensor.matmul(pAB[0:NCLS, 128:128 + BL], sb["wc2"][:], c1[:])
            pred = work.tile([NCLS, BL], f32, tag="pred", name="pred")
            nc.vector.tensor_scalar(
                pred[:], pAB[0:NCLS, 128:128 + BL], sb["bc2c"][:], None, OP.add
            )
            nc.sync.dma_start(out_dram[:], pred[:])

    nc.compile()
    return nc


def make_in_maps(inputs):
    shared = _prep_shared(
        inputs["W0"], inputs["b0"], inputs["W_in"], inputs["b_in"],
        inputs["W_h"], inputs["b_h"], inputs["W_out"], inputs["b_out"],
        inputs["Wc1"], inputs["bc1"], inputs["Wc2"], inputs["bc2"],
    )
    bc = np.asarray(inputs["batch_coeffs"], np.float32)
    in_maps = []
    for c in range(NCORES):
        x0t, dxh = _prep_percore(bc[c * BL:(c + 1) * BL])
        in_maps.append({**shared, "x0t": x0t, "dxh": dxh})
    return in_maps


_CACHED = {}


def kernel(**inputs):
    from concourse.bass_utils import run_bass_kernel_spmd

    if "nc" not in _CACHED:
        _CACHED["nc"] = build_nc()
    nc = _CACHED["nc"]
    in_maps = make_in_maps(inputs)
    res = run_bass_kernel_spmd(
        nc, in_maps, core_ids=list(range(NCORES)),
        trace=bool(int(os.environ.get("NCDE_TRACE", "0"))),
    )
    _CACHED["last_result"] = res
    out = np.zeros((B, NCLS), np.float32)
    for c in range(NCORES):
        out[c * BL:(c + 1) * BL, :] = res.results[c]["pred_t"].T
    return out

